# revision 23
# baseline (speedup 1.0000x reference)
"""Multi-head attention (B=2, S=2048, D=2048, H=16, hd=128) on 8 TRN2 NeuronCores.

Sharding: data-parallel over batch (2) x tensor-parallel over head groups (4).
Core c handles batch c//4 and heads [4*(c%4), 4*(c%4)+4). Each core computes
q/k/v projections for its 512 features, RoPE, full attention over S for its 4
heads, and a partial output projection y_partial = attn_local @ wo[:, cols].T.
Host sums the 4 partials per batch (no on-chip collectives).

All matmuls run in f16 with fp32 PSUM accumulation. The 1/sqrt(hd) score
scale is folded into wq host-side. RoPE pairs are split even/odd across the
partition dim by permuting wq/wk rows host-side, so RoPE is elementwise DVE
work against stacked [cos;cos] / [sin;sin] tables. Scores are computed
transposed ([k, q]) so softmax(exp)@V needs no on-chip transposes; the
softmax denominator is accumulated on DVE, all-reduced across partitions on
GpSimd, and divided out after the PV matmul.

Emission order is a software pipeline that keeps TensorE dense: k proj,
q proj with the first two attention score blocks interleaved, v proj (exp
hides under the v GEMM), then steady-state
[pv(b) | scores(b+2) | projection(finished q-chunk)].
"""

import numpy as np

B = 2
S = 2048
D = 2048
H = 16
HD = 128
P = 128
N_CORES = 8
H_LOC = 4          # heads per core
F = H_LOC * HD     # local features = 512
NCH = 4            # n-chunks of 512 over S
CH = S // NCH      # 512
DCH = D // P       # 16 contraction chunks
NT = S // P        # 16 row tiles

_F16 = np.float16


def _build_program():
    import concourse.bass_isa as bass_isa
    import concourse.mybir as mybir
    import concourse.tile as tile
    from concourse import bacc

    dt = mybir.dt
    nc = bacc.Bacc("TRN2", target_bir_lowering=False, debug=False,
                   num_devices=N_CORES)

    xT = nc.dram_tensor("xT", [D, S], dt.float16, kind="ExternalInput").ap()
    wqT = nc.dram_tensor("wqT", [D, F], dt.float16, kind="ExternalInput").ap()
    wkT = nc.dram_tensor("wkT", [D, F], dt.float16, kind="ExternalInput").ap()
    wvT = nc.dram_tensor("wvT", [D, F], dt.float16, kind="ExternalInput").ap()
    woT = nc.dram_tensor("woT", [F, D], dt.float16, kind="ExternalInput").ap()
    # stacked RoPE tables: [cos;cos] and [sin;sin]
    ct = nc.dram_tensor("ct", [P, S], dt.float16, kind="ExternalInput").ap()
    st = nc.dram_tensor("st", [P, S], dt.float16, kind="ExternalInput").ap()
    y = nc.dram_tensor("y", [S, D], dt.float32, kind="ExternalOutput").ap()

    xT3 = xT.rearrange("(o p) n -> p o n", p=P)      # [128, 16, 2048]
    wqT3 = wqT.rearrange("(o p) f -> p o f", p=P)    # [128, 16, 512]
    wkT3 = wkT.rearrange("(o p) f -> p o f", p=P)
    wvT3 = wvT.rearrange("(o p) f -> p o f", p=P)
    woT3 = woT.rearrange("(o p) n -> p o n", p=P)    # [128, 4, 2048]
    y3 = y.rearrange("(o p) n -> p o n", p=P)        # [128, 16, 2048]

    NB = NCH * H_LOC  # 16 attention blocks, b = qc*4 + h

    with tile.TileContext(nc) as tc:
        with (
            tc.tile_pool(name="persist", bufs=1) as pp,
            tc.tile_pool(name="xcp", bufs=3) as xcp,
        ):
            qTp = pp.tile([P, H_LOC, S], dt.float16, tag="qTp")
            kTp = pp.tile([P, H_LOC, S], dt.float16, tag="kTp")
            v_sb = pp.tile([P, NT, F], dt.float16, tag="v")
            wv_sb = pp.tile([P, DCH, F], dt.float16, tag="wv")
            wo_sb = pp.tile([P, H_LOC, D], dt.float16, tag="wo")

            # ---- phase 1: k and q projections + RoPE ---------------------
            with (
                tc.tile_pool(name="wp", bufs=1) as wp,
                tc.tile_pool(name="t2p", bufs=3) as t2p,
                tc.tile_pool(name="psg", bufs=1, space="PSUM") as psg,
            ):
                DH = DCH // 2
                wk_a = wp.tile([P, DH, F], dt.float16, tag="wk_a")
                wk_b = wp.tile([P, DH, F], dt.float16, tag="wk_b")
                wq_a = wp.tile([P, DH, F], dt.float16, tag="wq_a")
                wq_b = wp.tile([P, DH, F], dt.float16, tag="wq_b")
                ct_sb = wp.tile([P, S], dt.float16, tag="ct")
                st_sb = wp.tile([P, S], dt.float16, tag="st")

                # DMA issue order = need order: first half of wk + first x
                # chunk gate the very first matmul.
                nc.sync.dma_start(wk_a[:], wkT3[:, 0:DH, :])
                xc0 = xcp.tile([P, DCH, CH], dt.float16, tag="xc")
                nc.sync.dma_start(xc0[:], xT3[:, :, 0:CH])
                nc.sync.dma_start(wk_b[:], wkT3[:, DH:DCH, :])
                nc.sync.dma_start(ct_sb[:], ct[:])
                nc.sync.dma_start(st_sb[:], st[:])
                xc1 = xcp.tile([P, DCH, CH], dt.float16, tag="xc")
                nc.sync.dma_start(xc1[:], xT3[:, :, CH:2 * CH])
                nc.sync.dma_start(wq_a[:], wqT3[:, 0:DH, :])
                nc.sync.dma_start(wq_b[:], wqT3[:, DH:DCH, :])

                def proj_rope(w_halves, outT, nchunk, xc=None):
                    """One n-chunk of a q/k projection + RoPE into outT.
                    xc may be a whole [P, DCH, CH] tile or a (lo, hi) halves
                    tuple of [P, DH, CH] tiles."""
                    nsl = slice(nchunk * CH, (nchunk + 1) * CH)
                    if xc is None:
                        xc = xcp.tile([P, DCH, CH], dt.float16, tag="xc")
                        nc.sync.dma_start(xc[:], xT3[:, :, nsl])
                    xc_halves = xc if isinstance(xc, tuple) else (xc, xc)
                    whole = not isinstance(xc, tuple)
                    for h in range(H_LOC):
                        ps = psg.tile([P, CH], dt.float32, tag="gemm", bufs=3)
                        for dc in range(DCH):
                            w_sb = w_halves[dc // DH]
                            xc_sb = xc_halves[dc // DH]
                            xci = dc if whole else dc % DH
                            nc.tensor.matmul(
                                ps[:], w_sb[:, dc % DH, h * HD:(h + 1) * HD],
                                xc_sb[:, xci, :],
                                start=(dc == 0), stop=(dc == DCH - 1))
                        # RoPE: partitions 0:64 = even pairs e, 64:128 odd o:
                        #   out_e = e*c - o*s ; out_o = e*s + o*c
                        t1 = psg.tile([P, CH], dt.float32, tag="t1", bufs=2)
                        t2 = t2p.tile([P, CH], dt.float16, tag="t2")
                        nc.vector.tensor_mul(out=t1[:], in0=ps[:],
                                             in1=ct_sb[:, nsl])
                        nc.vector.tensor_mul(out=t2[:], in0=ps[:],
                                             in1=st_sb[:, nsl])
                        o_sl = outT[:, h, nsl]
                        nc.vector.tensor_sub(out=o_sl[0:64, :], in0=t1[0:64, :],
                                             in1=t2[64:128, :])
                        nc.vector.tensor_add(out=o_sl[64:128, :],
                                             in0=t2[0:64, :],
                                             in1=t1[64:128, :])

                kxc = {0: xc0, 1: xc1}
                for nchunk in range(NCH):
                    proj_rope((wk_a, wk_b), kTp, nchunk, xc=kxc.get(nchunk))
                nc.sync.dma_start(wv_sb[:], wvT3[:])
                vxc0 = xcp.tile([P, DCH, CH], dt.float16, tag="xc",
                                name="vxc0")
                nc.sync.dma_start(vxc0[:], xT3[:, :, 0:CH])
                for nchunk in range(NCH):
                    proj_rope((wq_a, wq_b), qTp, nchunk)

            # ---- phase 2: scores pipeline + v + pv + projection ----------
            with (
                tc.tile_pool(name="etp", bufs=16) as etp,
                tc.tile_pool(name="attnp", bufs=8) as attnp,
                tc.tile_pool(name="accp", bufs=3) as accp,
                tc.tile_pool(name="ytp", bufs=4) as ytp,
                tc.tile_pool(name="psc", bufs=1, space="PSUM") as psc,
            ):
                acc_of = {}
                from collections import deque
                sc_iters = deque()

                def scores_gen(b):
                    """Emit one score+exp+acc unit (2 matmuls) per yield, so
                    callers can interleave units with other TensorE work."""
                    qc, h = divmod(b, H_LOC)
                    qsl = slice(qc * CH, (qc + 1) * CH)
                    ets = []
                    acc = accp.tile([P, 2, CH], dt.float16, tag="acc")
                    acc_of[b] = (acc, ets)
                    for ktp in range(NT // 2):
                        ss = psc.tile([P, 2, CH], dt.float32, tag="ss", bufs=2)
                        for i in range(2):
                            kt = 2 * ktp + i
                            nc.tensor.matmul(
                                ss[:, i, :], kTp[:, h, kt * P:(kt + 1) * P],
                                qTp[:, h, qsl], start=True, stop=True)
                        et = etp.tile([P, 2, CH], dt.float16, tag="et")
                        nc.scalar.activation(
                            et[:], ss[:], mybir.ActivationFunctionType.Exp)
                        if ktp == 0:
                            nc.vector.tensor_copy(acc[:], et[:])
                        else:
                            nc.vector.tensor_add(out=acc[:], in0=acc[:],
                                                 in1=et[:])
                        ets.append(et)
                        yield

                def pump(n=1):
                    for _ in range(n):
                        while sc_iters:
                            try:
                                next(sc_iters[0])
                                break
                            except StopIteration:
                                sc_iters.popleft()

                allr_of = {}

                def emit_allred(b):
                    # denominator all-reduce, emitted one block ahead of its
                    # pv block: acc(b) is complete, GpSimd latency fully
                    # overlaps the preceding pv matmuls.
                    acc, _ = acc_of[b]
                    denom = accp.tile([P, CH], dt.float32, tag="den", bufs=2)
                    nc.vector.tensor_add(out=denom[:], in0=acc[:, 0, :],
                                         in1=acc[:, 1, :])
                    allr = accp.tile([P, CH], dt.float32, tag="allr", bufs=3)
                    nc.gpsimd.partition_all_reduce(
                        allr[:], denom[:], channels=P,
                        reduce_op=bass_isa.ReduceOp.add)
                    allr_of[b] = allr

                def pv_block(b, attn_cur):
                    qc, h = divmod(b, H_LOC)
                    hsl = slice(h * HD, (h + 1) * HD)
                    acc, ets = acc_of.pop(b)
                    allr = allr_of.pop(b)
                    pv = psc.tile([P, CH], dt.float32, tag="pv", bufs=2)
                    for ktp in range(NT // 2):
                        et = ets[ktp]
                        for i in range(2):
                            kt = 2 * ktp + i
                            nc.tensor.matmul(
                                pv[:], v_sb[:, kt, hsl], et[:, i, :],
                                start=(kt == 0), stop=(kt == NT - 1))
                        pump(1)
                    if b + 1 < NB:
                        emit_allred(b + 1)
                    rec = accp.tile([P, CH], dt.float32, tag="rec", bufs=2)
                    nc.vector.reciprocal_approx_fast(rec[:], allr[:])
                    nc.vector.tensor_mul(
                        out=attn_cur[:], in0=pv[:], in1=rec[:])

                def proj_chunk(qc, attn_heads, ntls=range(NCH)):
                    for ntl in ntls:
                        nt = qc * NCH + ntl
                        for half in range(2):
                            yt = ytp.tile([P, D // 2], dt.float32, tag="yt")
                            for i in range(2):
                                oc = half * 2 + i
                                py = psc.tile([P, CH], dt.float32, tag="py",
                                              bufs=2)
                                for h in range(H_LOC):
                                    nc.tensor.matmul(
                                        py[:],
                                        attn_heads[h][:, ntl * P:(ntl + 1) * P],
                                        wo_sb[:, h, oc * CH:(oc + 1) * CH],
                                        start=(h == 0), stop=(h == H_LOC - 1))
                                nc.scalar.activation(
                                    yt[:, i * CH:(i + 1) * CH], py[:],
                                    mybir.ActivationFunctionType.Copy)
                            nc.sync.dma_start(
                                y3[:, nt, half * D // 2:(half + 1) * D // 2],
                                yt[:])

                # v projection, with the first two score blocks pumped
                # in fine-grained units between v PSUM groups
                sc_iters.append(scores_gen(0))
                sc_iters.append(scores_gen(1))
                for nchunk in range(NCH):
                    nsl = slice(nchunk * CH, (nchunk + 1) * CH)
                    if nchunk == 0:
                        xc = vxc0
                    else:
                        xc = xcp.tile([P, DCH, CH], dt.float16, tag="xc")
                        nc.sync.dma_start(xc[:], xT3[:, :, nsl])
                    for nt in range(NCH):
                        ps = psc.tile([P, CH], dt.float32, tag="pv", bufs=2)
                        for dc in range(DCH):
                            nc.tensor.matmul(
                                ps[:], xc[:, dc, nt * P:(nt + 1) * P],
                                wv_sb[:, dc, :],
                                start=(dc == 0), stop=(dc == DCH - 1))
                        nc.scalar.activation(
                            v_sb[:, nchunk * NCH + nt, :], ps[:],
                            mybir.ActivationFunctionType.Copy)
                        pump(1)
                    if nchunk == 0:
                        nc.sync.dma_start(wo_sb[:], woT3[:])

                # steady state: [pv(b) | scores(b+2) units | proj(qc-1)]
                NB = NCH * H_LOC
                attn_hist = {}
                emit_allred(0)
                for b in range(NB):
                    qc, h = divmod(b, H_LOC)
                    if h == 0:
                        attn_hist[qc] = []
                    at = attnp.tile([P, CH], dt.float16, tag="attn",
                                    name=f"attn_{b}")
                    attn_hist[qc].append(at)
                    if b + 2 < NB:
                        sc_iters.append(scores_gen(b + 2))
                    pv_block(b, at)
                    if h == 0 and b > 0:
                        proj_chunk(qc - 1, attn_hist.pop(qc - 1))
                pump(100)
                proj_chunk(NCH - 1, attn_hist.pop(NCH - 1))

    nc.compile()
    return nc


_NC_CACHE = None


def _get_program():
    global _NC_CACHE
    if _NC_CACHE is None:
        _NC_CACHE = _build_program()
    return _NC_CACHE


def _rope_tables():
    scale = np.arange(0, HD, 2, dtype=np.float32) / HD
    inv_freq = 1.0 / (10000.0 ** scale)                 # [64]
    t = np.arange(S, dtype=np.float32)
    ang = np.outer(t, inv_freq)                         # [S, 64]
    cos = np.cos(ang).T.astype(np.float32)              # [64, S]
    sin = np.sin(ang).T.astype(np.float32)
    stk = lambda a: np.ascontiguousarray(
        np.concatenate([a, a], axis=0)).astype(_F16)    # [128, S]
    return stk(cos), stk(sin)


def prepare_in_maps(x, wq, wk, wv, wo):
    x = np.asarray(x, dtype=np.float32)
    wq = np.asarray(wq, dtype=np.float32) * np.float32(1.0 / np.sqrt(HD))
    wk = np.asarray(wk, dtype=np.float32)
    wv = np.asarray(wv, dtype=np.float32)
    wo = np.asarray(wo, dtype=np.float32)

    ct_t, st_t = _rope_tables()

    # even/odd RoPE permutation of rows within each head
    perm = np.concatenate([np.arange(0, HD, 2), np.arange(1, HD, 2)])

    xT = [np.ascontiguousarray(x[b].T).astype(_F16) for b in range(B)]

    in_maps = []
    for c in range(N_CORES):
        b, hg = divmod(c, H_LOC)
        heads = np.arange(hg * H_LOC, (hg + 1) * H_LOC)
        rows_qk = (heads[:, None] * HD + perm[None, :]).reshape(-1)  # [512]
        rows_nat = np.arange(hg * F, (hg + 1) * F)
        in_maps.append({
            "xT": xT[b],
            "wqT": np.ascontiguousarray(wq[rows_qk].T).astype(_F16),
            "wkT": np.ascontiguousarray(wk[rows_qk].T).astype(_F16),
            "wvT": np.ascontiguousarray(wv[rows_nat].T).astype(_F16),
            "woT": np.ascontiguousarray(wo[:, rows_nat].T).astype(_F16),
            "ct": ct_t, "st": st_t,
        })
    return in_maps


def combine_results(results):
    out = np.zeros((B, S, D), dtype=np.float32)
    for c, r in enumerate(results):
        out[c // H_LOC] += r["y"]
    return out


def kernel(x, wq, wk, wv, wo):
    from concourse.bass_utils import run_bass_kernel_spmd

    nc = _get_program()
    in_maps = prepare_in_maps(x, wq, wk, wv, wo)
    res = run_bass_kernel_spmd(nc, in_maps, core_ids=list(range(N_CORES)))
    return combine_results(res.results)


if __name__ == "__main__":
    rng = np.random.default_rng(0)
    ins = {
        "x": rng.standard_normal((B, S, D), dtype=np.float32),
        "wq": rng.standard_normal((D, D), dtype=np.float32) / np.sqrt(D),
        "wk": rng.standard_normal((D, D), dtype=np.float32) / np.sqrt(D),
        "wv": rng.standard_normal((D, D), dtype=np.float32) / np.sqrt(D),
        "wo": rng.standard_normal((D, D), dtype=np.float32) / np.sqrt(D),
    }
    out = kernel(**ins)
    print("out", out.shape, out.dtype, np.abs(out).max())
